# revision 7
# baseline (speedup 1.0000x reference)
"""Causal self-attention kernel for Trainium2, sharded over 8 NeuronCores.

Problem: x [4, 2048, 640] f32, w_qkv [1920, 640], w_proj [640, 640],
N_HEAD=10, head_dim=64.  out = proj(softmax(causal(q k^T / 8)) v).

Sharding: 40 (batch, head) pairs -> core c gets batch c//2 and the 5 heads
[5*(c%2), 5*(c%2)+5).  Each core computes a partial output
y_mine @ w_proj[:, mine].T  of shape [2048, 640]; the host sums core 2b
and 2b+1 to produce batch b.

Per-core device pipeline (all matmul operands bf16, fp32 PSUM accumulate):
  - host supplies x^T, packed/transposed weight slices, causal masks
  - qk projection -> qT/kT pair tiles [128, 2048] (head pairs stacked on
    partitions for row-packed score matmuls)
  - v projection -> v tiles [sk, 64+1] with a ones column (the ones column
    makes the AV matmul also produce the softmax denominator)
  - scores computed transposed: sT[sk, sq] = kT.T @ qT chunks, exp on
    ScalarE (scale=1/8 folded into the activation), causal mask via
    host-baked 0/1 bf16 masks on VectorE
  - AV: yT_aug[65, 512] += v_aug.T @ attnT  (row 64 = denominator)
  - 1/denom = exp(-log(denom)) on ScalarE, broadcast across partitions with
    a K=1 outer-product matmul, normalize on VectorE
  - output projection accumulated over head groups, DMA out as fp32
"""

import os
import sys

sys.path.insert(0, "/opt/trn_rl_repo")

import numpy as np
import ml_dtypes

import concourse.bass as bass
import concourse.bacc as bacc
import concourse.mybir as mybir
import concourse.tile as tile
from concourse.bass_utils import run_bass_kernel_spmd

BF16 = ml_dtypes.bfloat16
DT = mybir.dt

B, S, E = 4, 2048, 640
H, DH = 10, 64
N_CORES = 8
HPC = 5            # heads per core
SQ = 512           # sq chunk (scores moving free dim)
SKT = 128          # sk tile (scores psum partition dim)
NST = S // 128     # 16 s-tiles
NSQ = S // SQ      # 4 sq chunks

# qk column blocks inside the packed weight tensor wt [640, 960]:
# [qT01 | kT01 | qT23 | kT23 | qT4 | kT4 | v(5*64)]
QK_BLOCKS = [(0, 128), (128, 128), (256, 128), (384, 128), (512, 64), (576, 64)]
V_COL0 = 640

_NC_CACHE = None


def _build_program():
    nc = bacc.Bacc()
    xt_d = nc.dram_tensor("xt", [E, S], DT.bfloat16, kind="ExternalInput")
    wt_d = nc.dram_tensor("wt", [E, 960], DT.bfloat16, kind="ExternalInput")
    wp0_d = nc.dram_tensor("wp0", [128, E], DT.bfloat16, kind="ExternalInput")
    wp1_d = nc.dram_tensor("wp1", [128, E], DT.bfloat16, kind="ExternalInput")
    wp2_d = nc.dram_tensor("wp2", [64, E], DT.bfloat16, kind="ExternalInput")
    mk_d = nc.dram_tensor("masks", [128, 4 * SQ], DT.bfloat16, kind="ExternalInput")
    out_d = nc.dram_tensor("out", [S, E], DT.float32, kind="ExternalOutput")

    AF = mybir.ActivationFunctionType

    with tile.TileContext(nc) as tc:
        with (
            tc.tile_pool(name="persist", bufs=1) as pp,
            tc.tile_pool(name="work", bufs=4) as wk,
            tc.tile_pool(name="pmain", bufs=3, space="PSUM") as pm,
            tc.tile_pool(name="pyt", bufs=2, space="PSUM") as py,
        ):
            # ---- loads ----
            xts = []
            for e in range(5):
                xt_t = pp.tile([128, S], DT.bfloat16, name=f"xt{e}")
                nc.sync.dma_start(xt_t[:], xt_d[128 * e : 128 * e + 128, :])
                xts.append(xt_t)
            wts = []
            for e in range(5):
                wt_t = pp.tile([128, 960], DT.bfloat16, name=f"wt{e}")
                nc.sync.dma_start(wt_t[:], wt_d[128 * e : 128 * e + 128, :])
                wts.append(wt_t)
            wp0_t = pp.tile([128, E], DT.bfloat16, name="wp0")
            nc.sync.dma_start(wp0_t[:], wp0_d[:])
            wp1_t = pp.tile([128, E], DT.bfloat16, name="wp1")
            nc.sync.dma_start(wp1_t[:], wp1_d[:])
            wp2_t = pp.tile([64, E], DT.bfloat16, name="wp2")
            nc.sync.dma_start(wp2_t[:], wp2_d[:])
            mk_t = pp.tile([128, 4 * SQ], DT.bfloat16, name="mk")
            nc.sync.dma_start(mk_t[:], mk_d[:])

            ones_t = pp.tile([65, 64], DT.bfloat16, name="ones")
            nc.vector.memset(ones_t[:], 1.0)

            # ---- qk projection: qT/kT blocks [wid, 2048] (d-major) ----
            qk_sb = []
            for i, (c0, wid) in enumerate(QK_BLOCKS):
                qk_sb.append(pp.tile([wid, S], DT.bfloat16, name=f"qk{i}"))
            for i, (c0, wid) in enumerate(QK_BLOCKS):
                for sc in range(NSQ):
                    qkps = pm.tile([128, 1024], DT.float32, tag="slot", name="qkps")
                    pv = qkps[:wid, 0:SQ]
                    for e in range(5):
                        nc.tensor.matmul(
                            pv,
                            wts[e][:, c0 : c0 + wid],
                            xts[e][:, SQ * sc : SQ * sc + SQ],
                            start=(e == 0),
                            stop=(e == 4),
                        )
                    nc.vector.tensor_copy(qk_sb[i][:, SQ * sc : SQ * sc + SQ], pv)

            # ---- v projection: v_all [128(sk within tile), 5 heads * 16 st * 65]
            # layout per head h: columns [1040h, 1040h+1040), per s-tile t the
            # block [65t, 65t+64) holds v, column 65t+64 stays 1.0 (ones col).
            v_all = pp.tile([128, 5 * 1040], DT.bfloat16, name="v_all")
            nc.vector.memset(v_all[:], 1.0)
            for t in range(NST):
                vps = pm.tile([128, 1024], DT.float32, tag="slot", name="vps")
                pv = vps[:, 0:320]
                for e in range(5):
                    nc.tensor.matmul(
                        pv,
                        xts[e][:, 128 * t : 128 * t + 128],
                        wts[e][:, V_COL0 : V_COL0 + 320],
                        start=(e == 0),
                        stop=(e == 4),
                    )
                for h in range(5):
                    nc.vector.tensor_copy(
                        v_all[:, 1040 * h + 65 * t : 1040 * h + 65 * t + 64],
                        pv[:, 64 * h : 64 * h + 64],
                    )

            # ---- attention ----
            # pairs: (q block idx, k block idx, n heads, yt tile, head positions)
            yt01 = pp.tile([128, S], DT.bfloat16, name="yt01")
            yt23 = pp.tile([128, S], DT.bfloat16, name="yt23")
            yt4 = pp.tile([64, S], DT.bfloat16, name="yt4")
            pairs = [
                (0, 1, 2, yt01, (0, 1)),
                (2, 3, 2, yt23, (2, 3)),
                (4, 5, 1, yt4, (4,)),
            ]
            for qi, ki, nh, yt_t, hpos in pairs:
                qT, kT = qk_sb[qi], qk_sb[ki]
                for j in range(NSQ):
                    Ys = [
                        py.tile([65, SQ], DT.float32, tag="yt", name=f"Y{hi}")
                        for hi in range(nh)
                    ]
                    ngroups = 2 * j + 2  # skt in [0, 4j+4), groups of 2
                    for g in range(ngroups):
                        for hi in range(nh):
                            base = 64 * hi
                            Sc = pm.tile(
                                [128, 1024], DT.float32, tag="slot", name="Sc"
                            )
                            for m in range(2):
                                skt = 2 * g + m
                                nc.tensor.matmul(
                                    Sc[:, SQ * m : SQ * m + SQ],
                                    kT[base : base + 64, 128 * skt : 128 * skt + 128],
                                    qT[base : base + 64, SQ * j : SQ * j + SQ],
                                    start=True,
                                    stop=True,
                                    tile_position=(base, 0),
                                )
                            At = wk.tile([128, 1024], DT.bfloat16, tag="attn", name="At")
                            nc.scalar.activation(At[:], Sc[:], AF.Exp, scale=0.125)
                            for m in range(2):
                                skt = 2 * g + m
                                dpat = skt - 4 * j
                                if dpat >= 0:  # diagonal tile: causal mask
                                    nc.vector.tensor_mul(
                                        At[:, SQ * m : SQ * m + SQ],
                                        At[:, SQ * m : SQ * m + SQ],
                                        mk_t[:, SQ * dpat : SQ * dpat + SQ],
                                    )
                            for m in range(2):
                                skt = 2 * g + m
                                vcol = 1040 * hpos[hi] + 65 * skt
                                nc.tensor.matmul(
                                    Ys[hi][:, :],
                                    v_all[:, vcol : vcol + 65],
                                    At[:, SQ * m : SQ * m + SQ],
                                    start=(g == 0 and m == 0),
                                    stop=(g == ngroups - 1 and m == 1),
                                )
                    # normalize: row 64 of Y = denominator
                    for hi in range(nh):
                        Y = Ys[hi]
                        lnden = wk.tile([65, SQ], DT.float32, tag="lnden", name="lnden")
                        nc.scalar.activation(lnden[64:65, :], Y[64:65, :], AF.Ln)
                        recip = wk.tile([65, SQ], DT.bfloat16, tag="recip", name="recip")
                        nc.scalar.activation(
                            recip[64:65, :], lnden[64:65, :], AF.Exp, scale=-1.0
                        )
                        Bc = pm.tile([128, 1024], DT.float32, tag="slot", name="Bc")
                        nc.tensor.matmul(
                            Bc[0:64, 0:SQ],
                            ones_t[64:65, :],
                            recip[64:65, :],
                            start=True,
                            stop=True,
                            tile_position=(64, 0),
                        )
                        # DVE can read only one PSUM operand: stage the
                        # broadcast tile in SBUF before the normalize multiply
                        Bc_sb = wk.tile([64, SQ], DT.bfloat16, tag="bcsb", name="Bc_sb")
                        nc.vector.tensor_copy(Bc_sb[:], Bc[0:64, 0:SQ])
                        if hi == 0:
                            nc.vector.tensor_mul(
                                yt_t[0:64, SQ * j : SQ * j + SQ],
                                Y[0:64, :],
                                Bc_sb[:],
                            )
                        else:
                            ytmp = wk.tile([64, SQ], DT.bfloat16, tag="ytmp", name="ytmp")
                            nc.vector.tensor_mul(ytmp[:], Y[0:64, :], Bc_sb[:])
                            nc.sync.dma_start(
                                yt_t[64:128, SQ * j : SQ * j + SQ], ytmp[:]
                            )

            # ---- output projection: out[s, e] accumulated over 3 head groups ----
            for st in range(NST):
                O = pm.tile([128, 1024], DT.float32, tag="slot", name="O")
                groups = [
                    (yt01[:, 128 * st : 128 * st + 128], wp0_t),
                    (yt23[:, 128 * st : 128 * st + 128], wp1_t),
                    (yt4[:, 128 * st : 128 * st + 128], wp2_t),
                ]
                for c0, cw in ((0, 512), (512, 128)):
                    for gi, (yg, wg) in enumerate(groups):
                        nc.tensor.matmul(
                            O[:, c0 : c0 + cw],
                            yg,
                            wg[:, c0 : c0 + cw],
                            start=(gi == 0),
                            stop=(gi == 2),
                        )
                osb = wk.tile([128, E], DT.float32, tag="osb", name="osb")
                nc.vector.tensor_copy(osb[:], O[:, 0:E])
                nc.sync.dma_start(out_d[128 * st : 128 * st + 128, :], osb[:])

    nc.compile()
    return nc


def _get_nc():
    global _NC_CACHE
    if _NC_CACHE is None:
        _NC_CACHE = _build_program()
    return _NC_CACHE


def _make_masks():
    # mask d (d = skt - 4j in 0..3): [p, f] = 1.0 iff p + 128 d <= f
    p = np.arange(128)[:, None]
    f = np.arange(SQ)[None, :]
    cols = [(p + 128 * d <= f).astype(BF16) for d in range(4)]
    return np.concatenate(cols, axis=1)  # [128, 2048]


def _prep_core_inputs(x, w_qkv, w_proj):
    masks = _make_masks()
    in_maps = []
    for c in range(N_CORES):
        b, half = c // 2, c % 2
        hs = [HPC * half + i for i in range(HPC)]
        q = lambda h: w_qkv[DH * h : DH * h + DH]
        k = lambda h: w_qkv[E + DH * h : E + DH * h + DH]
        v = lambda h: w_qkv[2 * E + DH * h : 2 * E + DH * h + DH]
        wsel = np.concatenate(
            [
                q(hs[0]), q(hs[1]), k(hs[0]), k(hs[1]),
                q(hs[2]), q(hs[3]), k(hs[2]), k(hs[3]),
                q(hs[4]), k(hs[4]),
                v(hs[0]), v(hs[1]), v(hs[2]), v(hs[3]), v(hs[4]),
            ],
            axis=0,
        )  # [960, 640]
        wp = lambda h: w_proj[:, DH * h : DH * h + DH]  # [640, 64]
        in_maps.append(
            {
                "xt": np.ascontiguousarray(x[b].T).astype(BF16),
                "wt": np.ascontiguousarray(wsel.T).astype(BF16),
                "wp0": np.ascontiguousarray(
                    np.concatenate([wp(hs[0]), wp(hs[1])], axis=1).T
                ).astype(BF16),
                "wp1": np.ascontiguousarray(
                    np.concatenate([wp(hs[2]), wp(hs[3])], axis=1).T
                ).astype(BF16),
                "wp2": np.ascontiguousarray(wp(hs[4]).T).astype(BF16),
                "masks": masks,
            }
        )
    return in_maps


def _run(x, w_qkv, w_proj, trace=False, tmpdir=None):
    nc = _get_nc()
    in_maps = _prep_core_inputs(x, w_qkv, w_proj)
    res = run_bass_kernel_spmd(
        nc, in_maps, list(range(N_CORES)), trace=trace, tmpdir=tmpdir
    )
    out = np.empty((B, S, E), np.float32)
    for b in range(B):
        out[b] = res.results[2 * b]["out"] + res.results[2 * b + 1]["out"]
    return out, res


def kernel(x, w_qkv, w_proj):
    x = np.asarray(x, np.float32)
    w_qkv = np.asarray(w_qkv, np.float32)
    w_proj = np.asarray(w_proj, np.float32)
    out, _ = _run(x, w_qkv, w_proj, trace=False)
    return out


# revision 9
# speedup vs baseline: 1.0626x; 1.0626x over previous
"""Causal self-attention kernel for Trainium2, sharded over 8 NeuronCores.

Problem: x [4, 2048, 640] f32, w_qkv [1920, 640], w_proj [640, 640],
N_HEAD=10, head_dim=64.  out = proj(softmax(causal(q k^T / 8)) v).

Sharding: 40 (batch, head) pairs -> core c gets batch c//2 and the 5 heads
[5*(c%2), 5*(c%2)+5).  Each core computes a partial output
y_mine @ w_proj[:, mine].T  of shape [2048, 640]; the host sums core 2b
and 2b+1 to produce batch b.

Per-core device pipeline (all matmul operands bf16, fp32 PSUM accumulate):
  - host supplies x^T, packed/transposed weight slices, causal masks
  - qk projection -> qT/kT pair tiles [128, 2048] (head pairs stacked on
    partitions for row-packed score matmuls)
  - v projection -> v tiles [sk, 64+1] with a ones column (the ones column
    makes the AV matmul also produce the softmax denominator)
  - scores computed transposed: sT[sk, sq] = kT.T @ qT chunks, exp on
    ScalarE (scale=1/8 folded into the activation), causal mask via
    host-baked 0/1 bf16 masks on VectorE
  - AV: yT_aug[65, 512] += v_aug.T @ attnT  (row 64 = denominator)
  - 1/denom = exp(-ln(denom)) on ScalarE, broadcast across partitions with
    a K=1 outer-product matmul, normalize on VectorE
  - output projection accumulated over head groups, DMA out as fp32
"""

import os
import sys

sys.path.insert(0, "/opt/trn_rl_repo")

import numpy as np
import ml_dtypes

import bass_rust as _bass_rust
import concourse.bass as bass
import concourse.bacc as bacc
import concourse.mybir as mybir
import concourse.tile as tile
from concourse.bass_utils import run_bass_kernel_spmd
from concourse.hw_specs import get_activation_tables

BF16 = ml_dtypes.bfloat16
DT = mybir.dt

B, S, E = 4, 2048, 640
H, DH = 10, 64
N_CORES = 8
HPC = 5            # heads per core
SQ = 512           # sq chunk (scores moving free dim)
NST = S // 128     # 16 s-tiles
NSQ = S // SQ      # 4 sq chunks

# qk column blocks inside the packed weight tensor wt [640, 960]:
# [qT01 | kT01 | qT23 | kT23 | qT4 | kT4 | v(5*64)]
QK_BLOCKS = [(0, 128), (128, 128), (256, 128), (384, 128), (512, 64), (576, 64)]
V_COL0 = 640

_NC_CACHE = None


class _Bacc(bacc.Bacc):
    """Bacc with act-table selection pinned to the combined exp+ln set.

    The stock pass picks the first table set containing each activation's
    function; Exp resolves to exp_and_others while Ln resolves to
    natural_log_exp_and_others, so a kernel alternating Exp and Ln thrashes
    ACT_TABLE_LOAD (~1.3us each).  natural_log_exp_and_others contains both,
    so hide Exp/Ln from every other set and one load serves the kernel.
    """

    def insert_act_table_loads(self):
        has_activation = any(
            isinstance(i, mybir.InstActivation)
            for b in self.main_func.blocks
            for i in b.instructions
        )
        if not has_activation:
            return
        tables = get_activation_tables(self.m.arch)
        AF = mybir.ActivationFunctionType
        combined = tables.get("natural_log_exp_and_others")
        if combined and AF.Exp in combined and AF.Ln in combined:
            for name, fns in tables.items():
                if name != "natural_log_exp_and_others":
                    fns.discard(AF.Exp)
                    fns.discard(AF.Ln)
        _bass_rust.insert_act_table_loads(self, list(tables.items()))


def _build_program():
    nc = _Bacc()
    xt_d = nc.dram_tensor("xt", [E, S], DT.bfloat16, kind="ExternalInput")
    wt_d = nc.dram_tensor("wt", [E, 960], DT.bfloat16, kind="ExternalInput")
    wp0_d = nc.dram_tensor("wp0", [128, E], DT.bfloat16, kind="ExternalInput")
    wp1_d = nc.dram_tensor("wp1", [128, E], DT.bfloat16, kind="ExternalInput")
    wp2_d = nc.dram_tensor("wp2", [64, E], DT.bfloat16, kind="ExternalInput")
    mk_d = nc.dram_tensor("masks", [128, 4 * SQ], DT.bfloat16, kind="ExternalInput")
    out_d = nc.dram_tensor("out", [S, E], DT.float32, kind="ExternalOutput")

    AF = mybir.ActivationFunctionType

    with tile.TileContext(nc) as tc:
        with (
            tc.tile_pool(name="persist", bufs=1) as pp,
            tc.tile_pool(name="work", bufs=6) as wk,
            tc.tile_pool(name="pmain", bufs=4, space="PSUM") as pm,
            tc.tile_pool(name="pyt", bufs=4, space="PSUM") as py,
        ):
            # ---- loads ----
            xts = []
            for e in range(5):
                xt_t = pp.tile([128, S], DT.bfloat16, name=f"xt{e}")
                nc.sync.dma_start(xt_t[:], xt_d[128 * e : 128 * e + 128, :])
                xts.append(xt_t)
            wts = []
            for e in range(5):
                wt_t = pp.tile([128, 960], DT.bfloat16, name=f"wt{e}")
                nc.sync.dma_start(wt_t[:], wt_d[128 * e : 128 * e + 128, :])
                wts.append(wt_t)
            wp0_t = pp.tile([128, E], DT.bfloat16, name="wp0")
            nc.sync.dma_start(wp0_t[:], wp0_d[:])
            wp1_t = pp.tile([128, E], DT.bfloat16, name="wp1")
            nc.sync.dma_start(wp1_t[:], wp1_d[:])
            wp2_t = pp.tile([64, E], DT.bfloat16, name="wp2")
            nc.sync.dma_start(wp2_t[:], wp2_d[:])
            mk_t = pp.tile([128, 4 * SQ], DT.bfloat16, name="mk")
            nc.sync.dma_start(mk_t[:], mk_d[:])

            ones_t = pp.tile([65, 64], DT.bfloat16, name="ones")
            nc.vector.memset(ones_t[:], 1.0)

            # ---- qk projection: qT/kT blocks [wid, 2048] (d-major) ----
            qk_sb = []
            for i, (c0, wid) in enumerate(QK_BLOCKS):
                qk_sb.append(pp.tile([wid, S], DT.bfloat16, name=f"qk{i}"))
            for i, (c0, wid) in enumerate(QK_BLOCKS):
                for sc in range(NSQ):
                    qkps = pm.tile([128, SQ], DT.float32, tag="slot", name="qkps")
                    pv = qkps[:wid, :]
                    for e in range(5):
                        nc.tensor.matmul(
                            pv,
                            wts[e][:, c0 : c0 + wid],
                            xts[e][:, SQ * sc : SQ * sc + SQ],
                            start=(e == 0),
                            stop=(e == 4),
                        )
                    nc.vector.tensor_copy(qk_sb[i][:, SQ * sc : SQ * sc + SQ], pv)

            # ---- v projection: v_all [128(sk within tile), 5 heads * 16 st * 65]
            # layout per head h: columns [1040h, 1040h+1040), per s-tile t the
            # block [65t, 65t+64) holds v, column 65t+64 stays 1.0 (ones col).
            v_all = pp.tile([128, 5 * 1040], DT.bfloat16, name="v_all")
            nc.vector.memset(v_all[:], 1.0)
            for t in range(NST):
                vps = pm.tile([128, SQ], DT.float32, tag="slot", name="vps")
                pv = vps[:, 0:320]
                for e in range(5):
                    nc.tensor.matmul(
                        pv,
                        xts[e][:, 128 * t : 128 * t + 128],
                        wts[e][:, V_COL0 : V_COL0 + 320],
                        start=(e == 0),
                        stop=(e == 4),
                    )
                for h in range(5):
                    nc.vector.tensor_copy(
                        v_all[:, 1040 * h + 65 * t : 1040 * h + 65 * t + 64],
                        pv[:, 64 * h : 64 * h + 64],
                    )

            # ---- attention ----
            yt01 = pp.tile([128, S], DT.bfloat16, name="yt01")
            yt23 = pp.tile([128, S], DT.bfloat16, name="yt23")
            yt4 = pp.tile([64, S], DT.bfloat16, name="yt4")
            pairs = [
                (0, 1, 2, yt01, (0, 1)),
                (2, 3, 2, yt23, (2, 3)),
                (4, 5, 1, yt4, (4,)),
            ]
            for qi, ki, nh, yt_t, hpos in pairs:
                qT, kT = qk_sb[qi], qk_sb[ki]
                for j in range(NSQ):
                    Ys = [
                        py.tile([65, SQ], DT.float32, tag="yt", name=f"Y{hi}")
                        for hi in range(nh)
                    ]
                    nskt = 4 * j + 4
                    for skt in range(nskt):
                        for hi in range(nh):
                            base = 64 * hi
                            Sc = pm.tile([128, SQ], DT.float32, tag="slot", name="Sc")
                            nc.tensor.matmul(
                                Sc[:],
                                kT[base : base + 64, 128 * skt : 128 * skt + 128],
                                qT[base : base + 64, SQ * j : SQ * j + SQ],
                                start=True,
                                stop=True,
                                tile_position=(base, 0),
                            )
                            At = wk.tile([128, SQ], DT.bfloat16, tag="attn", name="At")
                            nc.scalar.activation(At[:], Sc[:], AF.Exp, scale=0.125)
                            dpat = skt - 4 * j
                            if dpat >= 0:  # diagonal tile: causal mask
                                nc.vector.tensor_mul(
                                    At[:],
                                    At[:],
                                    mk_t[:, SQ * dpat : SQ * dpat + SQ],
                                )
                            vcol = 1040 * hpos[hi] + 65 * skt
                            nc.tensor.matmul(
                                Ys[hi][:, :],
                                v_all[:, vcol : vcol + 65],
                                At[:],
                                start=(skt == 0),
                                stop=(skt == nskt - 1),
                            )
                    # normalize: row 64 of Y = denominator
                    for hi in range(nh):
                        Y = Ys[hi]
                        lnden = wk.tile(
                            [65, SQ], DT.float32, tag="lnden", name="lnden", bufs=2
                        )
                        nc.scalar.activation(lnden[64:65, :], Y[64:65, :], AF.Ln)
                        recip = wk.tile(
                            [65, SQ], DT.bfloat16, tag="recip", name="recip", bufs=2
                        )
                        nc.scalar.activation(
                            recip[64:65, :], lnden[64:65, :], AF.Exp, scale=-1.0
                        )
                        Bc = pm.tile([128, SQ], DT.float32, tag="slot", name="Bc")
                        nc.tensor.matmul(
                            Bc[0:64, :],
                            ones_t[64:65, :],
                            recip[64:65, :],
                            start=True,
                            stop=True,
                            tile_position=(64, 0),
                        )
                        # DVE can read only one PSUM operand: stage the
                        # broadcast tile in SBUF before the normalize multiply
                        Bc_sb = wk.tile(
                            [64, SQ], DT.bfloat16, tag="bcsb", name="Bc_sb", bufs=2
                        )
                        nc.vector.tensor_copy(Bc_sb[:], Bc[0:64, :])
                        if hi == 0:
                            nc.vector.tensor_mul(
                                yt_t[0:64, SQ * j : SQ * j + SQ],
                                Y[0:64, :],
                                Bc_sb[:],
                            )
                        else:
                            ytmp = wk.tile(
                                [64, SQ], DT.bfloat16, tag="ytmp", name="ytmp", bufs=2
                            )
                            nc.vector.tensor_mul(ytmp[:], Y[0:64, :], Bc_sb[:])
                            nc.sync.dma_start(
                                yt_t[64:128, SQ * j : SQ * j + SQ], ytmp[:]
                            )

            # ---- output projection: out[s, e] accumulated over 3 head groups ----
            # PSUM is fully budgeted (slot 4 + yt 4 banks), so the [128, 640]
            # output borrows one slot-tag bank for cols 0:512 and one yt-tag
            # bank for cols 512:640.
            for st in range(NST):
                O1 = pm.tile([128, SQ], DT.float32, tag="slot", name="O1")
                O2 = py.tile([128, 128], DT.float32, tag="yt", name="O2")
                groups = [
                    (yt01[:, 128 * st : 128 * st + 128], wp0_t),
                    (yt23[:, 128 * st : 128 * st + 128], wp1_t),
                    (yt4[:, 128 * st : 128 * st + 128], wp2_t),
                ]
                for gi, (yg, wg) in enumerate(groups):
                    nc.tensor.matmul(
                        O1[:], yg, wg[:, 0:512], start=(gi == 0), stop=(gi == 2)
                    )
                for gi, (yg, wg) in enumerate(groups):
                    nc.tensor.matmul(
                        O2[:], yg, wg[:, 512:640], start=(gi == 0), stop=(gi == 2)
                    )
                osb = wk.tile([128, E], DT.float32, tag="osb", name="osb", bufs=3)
                nc.vector.tensor_copy(osb[:, 0:512], O1[:])
                nc.vector.tensor_copy(osb[:, 512:640], O2[:])
                nc.sync.dma_start(out_d[128 * st : 128 * st + 128, :], osb[:])

    nc.compile()
    return nc


def _get_nc():
    global _NC_CACHE
    if _NC_CACHE is None:
        _NC_CACHE = _build_program()
    return _NC_CACHE


def _make_masks():
    # mask d (d = skt - 4j in 0..3): [p, f] = 1.0 iff p + 128 d <= f
    p = np.arange(128)[:, None]
    f = np.arange(SQ)[None, :]
    cols = [(p + 128 * d <= f).astype(BF16) for d in range(4)]
    return np.concatenate(cols, axis=1)  # [128, 2048]


def _prep_core_inputs(x, w_qkv, w_proj):
    masks = _make_masks()
    in_maps = []
    for c in range(N_CORES):
        b, half = c // 2, c % 2
        hs = [HPC * half + i for i in range(HPC)]
        q = lambda h: w_qkv[DH * h : DH * h + DH]
        k = lambda h: w_qkv[E + DH * h : E + DH * h + DH]
        v = lambda h: w_qkv[2 * E + DH * h : 2 * E + DH * h + DH]
        wsel = np.concatenate(
            [
                q(hs[0]), q(hs[1]), k(hs[0]), k(hs[1]),
                q(hs[2]), q(hs[3]), k(hs[2]), k(hs[3]),
                q(hs[4]), k(hs[4]),
                v(hs[0]), v(hs[1]), v(hs[2]), v(hs[3]), v(hs[4]),
            ],
            axis=0,
        )  # [960, 640]
        wp = lambda h: w_proj[:, DH * h : DH * h + DH]  # [640, 64]
        in_maps.append(
            {
                "xt": np.ascontiguousarray(x[b].T).astype(BF16),
                "wt": np.ascontiguousarray(wsel.T).astype(BF16),
                "wp0": np.ascontiguousarray(
                    np.concatenate([wp(hs[0]), wp(hs[1])], axis=1).T
                ).astype(BF16),
                "wp1": np.ascontiguousarray(
                    np.concatenate([wp(hs[2]), wp(hs[3])], axis=1).T
                ).astype(BF16),
                "wp2": np.ascontiguousarray(wp(hs[4]).T).astype(BF16),
                "masks": masks,
            }
        )
    return in_maps


def _run(x, w_qkv, w_proj, trace=False, tmpdir=None):
    nc = _get_nc()
    in_maps = _prep_core_inputs(x, w_qkv, w_proj)
    res = run_bass_kernel_spmd(
        nc, in_maps, list(range(N_CORES)), trace=trace, tmpdir=tmpdir
    )
    out = np.empty((B, S, E), np.float32)
    for b in range(B):
        out[b] = res.results[2 * b]["out"] + res.results[2 * b + 1]["out"]
    return out, res


def kernel(x, w_qkv, w_proj):
    x = np.asarray(x, np.float32)
    w_qkv = np.asarray(w_qkv, np.float32)
    w_proj = np.asarray(w_proj, np.float32)
    out, _ = _run(x, w_qkv, w_proj, trace=False)
    return out


if __name__ == "__main__":
    import jax

    jax.config.update("jax_platforms", "cpu")
    rng = np.random.default_rng(0)
    x = rng.standard_normal((B, S, E), dtype=np.float32)
    w_qkv = rng.standard_normal((3 * E, E), dtype=np.float32) / np.sqrt(E)
    w_proj = rng.standard_normal((E, E), dtype=np.float32) / np.sqrt(E)
    out = kernel(x, w_qkv, w_proj)
    print("out", out.shape, out.dtype, float(np.abs(out).max()))
